# revision 11
# baseline (speedup 1.0000x reference)
"""Ewald reciprocal-space sum on 8 Trainium2 NeuronCores.

Math: for each system b, S(k) = sum_n q_n e^{i k.r_n} over the static
integer k-grid n in [-10,10]^3, k = n @ G, G = 2*pi*inv(cell)^T.
Key identity: k.r = n1*phi1 + n2*phi2 + n3*phi3 with phi_d = G_d . r,
so e^{i k.r} factorizes into per-dimension phase tables. Only the
n1 >= 0 half-grid is needed (hemisphere mask kills n1 < 0).

Device work per core (SPMD, core c owns half the atoms of system c//2):
  - phases phi'_d = frac((r @ inv(cell))_d) come in pre-reduced (turns)
  - theta'[j,d] = j * phi'_d  for j in [-10..10]        (DVE)
  - reduce mod 1 into [0,1) turn space                  (DVE/GPSIMD)
  - sin/cos via ACT Sin(2*pi*t - pi)                    (ACT)
  - pair table A = e^{i(n2*phi2+n3*phi3)}  [atoms,441]  (ACT)
  - S partial = (q*e^{i n1 phi1})^T @ A via 2 PSUM-accumulated
    matmuls per 128-atom chunk                          (PE)
Host: tiny O(B*K) weight mask + final reduction (exactly mirrors the
reference), summing partial S across the core pair before squaring.
"""

import numpy as np

# ---- problem constants (hardcoded per contract) ----
B = 4
N_PER = 2000
NK = 10                      # k-grid extent: n in [-NK, NK]
NJ = 2 * NK + 1              # 21
NPAIR = NJ * NJ              # 441
NH = NK + 1                  # 11 non-negative n1 values
DL = 2.0
SIGMA = 1.0
EPS = 1e-6
NORM = 90.0474
TWOPI = 2.0 * np.pi

MAGIC = 12582912.0           # 1.5 * 2**23: fp32 round-to-nearest trick

N_CORES = 8
CORES_PER_SYS = 2
ATOMS_PER_CORE = (B * N_PER) // N_CORES     # 1000
CHUNKS = 8                                  # ceil(1000/128)
PADN = CHUNKS * 128                         # 1024

_CACHE = {}


def _build_nc():
    import concourse.bacc as bacc
    import concourse.mybir as mybir
    import concourse.tile as tile

    f32 = mybir.dt.float32
    Alu = mybir.AluOpType
    Act = mybir.ActivationFunctionType

    nc = bacc.Bacc(None, target_bir_lowering=False)

    phi_in = nc.dram_tensor("phi", [128, CHUNKS * 3], f32, kind="ExternalInput")
    q_in = nc.dram_tensor("qv", [128, CHUNKS], f32, kind="ExternalInput")
    sout = nc.dram_tensor("sout", [2 * NH, 2 * NPAIR], f32, kind="ExternalOutput")

    # j values, d-major blocks of 21: col = d*21 + (j+10)
    jdat = np.tile(np.arange(-NK, NK + 1, dtype=np.float32), (128, 3))
    jrow = nc.inline_tensor(jdat, name="jrow")

    with tile.TileContext(nc) as tc:
        with (
            tc.tile_pool(name="const", bufs=1) as cp,
            tc.tile_pool(name="work", bufs=3) as wp,
            tc.tile_pool(name="psum", bufs=1, space="PSUM") as pp,
        ):
            consts = cp.tile([128, 1], f32)
            nc.gpsimd.memset(consts[:, 0:1], -TWOPI)
            cm2pi = consts[:, 0:1]
            jt = cp.tile([128, 3 * NJ], f32)
            nc.sync.dma_start(out=jt[:], in_=jrow[:])
            phit = cp.tile([128, CHUNKS * 3], f32)
            nc.sync.dma_start(out=phit[:], in_=phi_in[:])
            qt = cp.tile([128, CHUNKS], f32)
            nc.sync.dma_start(out=qt[:], in_=q_in[:])

            ps_r = pp.tile([2 * NH, NPAIR], f32)
            ps_i = pp.tile([2 * NH, NPAIR], f32)

            for t in range(CHUNKS):
                # theta' = j * phi' (turns), all 3 dims: [128, 63]
                th = wp.tile([128, 3 * NJ], f32)
                phi3 = phit[:, 3 * t : 3 * t + 3]
                nc.vector.tensor_tensor(
                    out=th[:].rearrange("p (d j) -> p d j", d=3),
                    in0=phi3.unsqueeze(2).broadcast_to([128, 3, NJ]),
                    in1=jt[:].rearrange("p (d j) -> p d j", d=3),
                    op=Alu.mult,
                )
                # F = round(theta') - theta' = -frac_centered(theta'), in
                # [-0.5, 0.5] turns; magic-number round (exact for |x|<2^22)
                t1 = wp.tile([128, 3 * NJ], f32)
                nc.vector.tensor_scalar(
                    out=t1[:], in0=th[:], scalar1=MAGIC, scalar2=None, op0=Alu.add
                )
                F = wp.tile([128, 3 * NJ], f32)
                nc.vector.scalar_tensor_tensor(
                    out=F[:], in0=t1[:], scalar=-MAGIC, in1=th[:],
                    op0=Alu.add, op1=Alu.subtract,
                )
                # pair sum: un = F2 (+) F3 = -(psi') mod 1, in [-1, 1]
                un = wp.tile([128, NPAIR], f32)
                nc.vector.tensor_tensor(
                    out=un[:].rearrange("p (a b) -> p a b", a=NJ),
                    in0=F[:, NJ : 2 * NJ].unsqueeze(2).broadcast_to([128, NJ, NJ]),
                    in1=F[:, 2 * NJ : 3 * NJ].unsqueeze(1).broadcast_to([128, NJ, NJ]),
                    op=Alu.add,
                )
                # V holds all Sin inputs (turns in [-0.5,0.5], sign-flipped):
                # [A_i src | A_r src | c1 src | s1 src]; one fused ACT
                V = wp.tile([128, 2 * NPAIR + 2 * NH], f32)
                F1 = F[:, NK : NJ]
                nc.vector.add_range_wrap(
                    out=V[:, 0:NPAIR], in_=un[:], shift=0.0, bound=0.5, period=1.0
                )
                nc.vector.add_range_wrap(
                    out=V[:, NPAIR : 2 * NPAIR], in_=un[:], shift=-0.25,
                    bound=0.5, period=1.0,
                )
                nc.vector.add_range_wrap(
                    out=V[:, 2 * NPAIR : 2 * NPAIR + NH], in_=F1, shift=-0.25,
                    bound=0.5, period=1.0,
                )
                nc.vector.tensor_copy(
                    out=V[:, 2 * NPAIR + NH : 2 * NPAIR + 2 * NH], in_=F1
                )
                # Sin(-2pi * v): cols 0:441 A_i, 441:882 A_r, then c1, s1
                AA = wp.tile([128, 2 * NPAIR + 2 * NH], f32)
                nc.scalar.activation(
                    out=AA[:], in_=V[:], func=Act.Sin, bias=0.0, scale=cm2pi
                )
                # lhsT = [q*c1 | q*s1]  [128, 22]
                lhsT = wp.tile([128, 2 * NH], f32)
                qcol = qt[:, t : t + 1]
                nc.vector.tensor_scalar(
                    out=lhsT[:, 0:NH], in0=AA[:, 2 * NPAIR : 2 * NPAIR + NH],
                    scalar1=qcol, scalar2=None, op0=Alu.mult,
                )
                nc.vector.tensor_scalar(
                    out=lhsT[:, NH : 2 * NH],
                    in0=AA[:, 2 * NPAIR + NH : 2 * NPAIR + 2 * NH],
                    scalar1=qcol, scalar2=None, op0=Alu.mult,
                )
                # S partials: rows 0:11 = a^T X, rows 11:22 = b^T X
                nc.tensor.matmul(
                    out=ps_r[:], lhsT=lhsT[:], rhs=AA[:, NPAIR : 2 * NPAIR],
                    start=(t == 0), stop=(t == CHUNKS - 1),
                )
                nc.tensor.matmul(
                    out=ps_i[:], lhsT=lhsT[:], rhs=AA[:, 0:NPAIR],
                    start=(t == 0), stop=(t == CHUNKS - 1),
                )

            # PSUM -> SBUF -> DRAM (combine happens on host)
            so = wp.tile([2 * NH, 2 * NPAIR], f32)
            nc.vector.tensor_copy(out=so[:, 0:NPAIR], in_=ps_r[:])
            nc.vector.tensor_copy(out=so[:, NPAIR : 2 * NPAIR], in_=ps_i[:])
            nc.sync.dma_start(out=sout[:], in_=so[:])

    nc.compile()
    return nc


def _get_nc():
    if "nc" not in _CACHE:
        _CACHE["nc"] = _build_nc()
    return _CACHE["nc"]


def _host_inputs(q, r, cell):
    """Per-core phi (reduced turns) and q in SBUF layout."""
    in_maps = []
    for c in range(N_CORES):
        b = c // CORES_PER_SYS
        half = c % CORES_PER_SYS
        lo = b * N_PER + half * ATOMS_PER_CORE
        rs = r[lo : lo + ATOMS_PER_CORE].astype(np.float64)
        qs = q[lo : lo + ATOMS_PER_CORE, 0].astype(np.float32)
        minv = np.linalg.inv(cell[b].astype(np.float64))
        phi = (rs @ minv) % 1.0                      # [1000, 3] turns in [0,1)
        phi_p = np.zeros((PADN, 3), np.float32)
        phi_p[:ATOMS_PER_CORE] = phi.astype(np.float32)
        q_p = np.zeros((PADN,), np.float32)
        q_p[:ATOMS_PER_CORE] = qs
        # atom (t*128+p) -> [p, t*3+d] / [p, t]
        phi_l = np.ascontiguousarray(
            phi_p.reshape(CHUNKS, 128, 3).transpose(1, 0, 2).reshape(128, CHUNKS * 3)
        )
        q_l = np.ascontiguousarray(q_p.reshape(CHUNKS, 128).T)
        in_maps.append({"phi": phi_l, "qv": q_l})
    return in_maps


def _host_weights(cell):
    """w[b, n1(0..10), n2, n3] = mask * 2 * kfac / V, mirroring reference."""
    k_sq_max = (TWOPI / DL) ** 2
    sigma_sq_half = SIGMA ** 2 / 2.0
    rng = np.arange(-NK, NK + 1, dtype=np.float64)
    n1, n2, n3 = np.meshgrid(rng[NK:], rng, rng, indexing="ij")  # n1 >= 0
    nvec = np.stack([n1.ravel(), n2.ravel(), n3.ravel()], axis=1)  # [NH*441, 3]
    hemi = (
        (nvec[:, 0] > 0)
        | ((nvec[:, 0] == 0) & (nvec[:, 1] > 0))
        | ((nvec[:, 0] == 0) & (nvec[:, 1] == 0) & (nvec[:, 2] > 0))
    )
    ws = []
    for b in range(B):
        cb = cell[b].astype(np.float64)
        G = TWOPI * np.linalg.inv(cb).T
        kvec = nvec @ G
        k_sq = np.sum(kvec ** 2, axis=1)
        mask = (k_sq > 0) & (k_sq <= k_sq_max) & hemi
        kfac = np.exp(-sigma_sq_half * k_sq) / (k_sq + EPS)
        vol = np.linalg.det(cb)
        ws.append(np.where(mask, 2.0 * kfac, 0.0) / vol)
    return np.stack(ws).reshape(B, NH, NPAIR)


def kernel(q, r, cell, batch):
    from concourse.bass_utils import run_bass_kernel_spmd

    q = np.asarray(q)
    r = np.asarray(r)
    cell = np.asarray(cell)

    nc = _get_nc()
    in_maps = _host_inputs(q, r, cell)
    res = run_bass_kernel_spmd(nc, in_maps, core_ids=list(range(N_CORES))).results

    w = _host_weights(cell)
    pot = np.zeros(B, np.float64)
    for b in range(B):
        s_r = np.zeros((NH, NPAIR), np.float64)
        s_i = np.zeros_like(s_r)
        for half in range(CORES_PER_SYS):
            o = res[b * CORES_PER_SYS + half]["sout"].astype(np.float64)
            P, Q = o[0:NH, 0:NPAIR], o[NH : 2 * NH, 0:NPAIR]
            R, T = o[0:NH, NPAIR:], o[NH : 2 * NH, NPAIR:]
            s_r += P - T
            s_i += R + Q
        s_sq = s_r ** 2 + s_i ** 2
        qb = q[b * N_PER : (b + 1) * N_PER, 0].astype(np.float64)
        self_e = np.sum(qb ** 2) / (SIGMA * TWOPI ** 1.5)
        pot[b] = (np.sum(w[b] * s_sq) - self_e) * NORM
    return pot.astype(np.float32)


# revision 17
# speedup vs baseline: 1.0683x; 1.0683x over previous
"""Ewald reciprocal-space sum on 8 Trainium2 NeuronCores.

Math: for each system b, S(k) = sum_n q_n e^{i k.r_n} over the static
integer k-grid n in [-10,10]^3, k = n @ G, G = 2*pi*inv(cell)^T.
Key identity: k.r = n1*phi1 + n2*phi2 + n3*phi3 with phi_d = G_d . r,
so e^{i k.r} factorizes into per-dimension phase tables. Only the
n1 >= 0 half-grid is needed (hemisphere mask kills n1 < 0).

Device work per core (SPMD, core c owns half the atoms of system c//2):
  - phases phi'_d = frac((r @ inv(cell))_d) come in pre-reduced (turns)
  - theta'[j,d] = j * phi'_d  for j in [-10..10]        (DVE)
  - reduce mod 1 into [0,1) turn space                  (DVE/GPSIMD)
  - sin/cos via ACT Sin(2*pi*t - pi)                    (ACT)
  - pair table A = e^{i(n2*phi2+n3*phi3)}  [atoms,441]  (ACT)
  - S partial = (q*e^{i n1 phi1})^T @ A via 2 PSUM-accumulated
    matmuls per 128-atom chunk                          (PE)
Host: tiny O(B*K) weight mask + final reduction (exactly mirrors the
reference), summing partial S across the core pair before squaring.
"""

import numpy as np

# ---- problem constants (hardcoded per contract) ----
B = 4
N_PER = 2000
NK = 10                      # k-grid extent: n in [-NK, NK]
NJ = 2 * NK + 1              # 21
NPAIR = NJ * NJ              # 441
NH = NK + 1                  # 11 non-negative n1 values
DL = 2.0
SIGMA = 1.0
EPS = 1e-6
NORM = 90.0474
TWOPI = 2.0 * np.pi

MAGIC = 12582912.0           # 1.5 * 2**23: fp32 round-to-nearest trick
NPP = NPAIR + 1              # 442: fp32r matmul needs even free counts

N_CORES = 8
CORES_PER_SYS = 2
ATOMS_PER_CORE = (B * N_PER) // N_CORES     # 1000
CHUNKS = 8                                  # ceil(1000/128)
PADN = CHUNKS * 128                         # 1024

_CACHE = {}


def _build_nc():
    import concourse.bacc as bacc
    import concourse.mybir as mybir
    import concourse.tile as tile

    f32 = mybir.dt.float32
    Alu = mybir.AluOpType
    Act = mybir.ActivationFunctionType

    f32r = mybir.dt.float32r
    nc = bacc.Bacc(None, target_bir_lowering=False)

    # one input tensor: cols 0:24 = phi (chunk-major, 3/chunk), 24:32 = q
    inp = nc.dram_tensor("inp", [128, 3 * CHUNKS + CHUNKS], f32, kind="ExternalInput")
    sout = nc.dram_tensor("sout", [2 * NH, 2 * NPP], f32, kind="ExternalOutput")

    # j values (d-major blocks of 21, col = d*21 + j+10) + col 63 = -2pi
    jdat = np.concatenate(
        [
            np.tile(np.arange(-NK, NK + 1, dtype=np.float32), (128, 3)),
            np.full((128, 1), -TWOPI, np.float32),
        ],
        axis=1,
    )
    jrow = nc.inline_tensor(jdat, name="jrow")

    NW = 3 * NJ                      # 63 cols per chunk in F/th tiles
    NV = 4 * NPP + 4 * NH            # 1812 cols in fused ACT input per pair

    with tile.TileContext(nc) as tc:
        with (
            tc.tile_pool(name="const", bufs=1) as cp,
            tc.tile_pool(name="work", bufs=3) as wp,
            tc.tile_pool(name="psum", bufs=1, space="PSUM") as pp,
        ):
            jt = cp.tile([128, NW + 1], f32)
            nc.sync.dma_start(out=jt[:], in_=jrow[:])
            cm2pi = jt[:, NW : NW + 1]
            it = cp.tile([128, 4 * CHUNKS], f32)
            nc.sync.dma_start(out=it[:], in_=inp[:])

            ps_r = pp.tile([2 * NH, NPP], f32)
            ps_i = pp.tile([2 * NH, NPP], f32)

            # stage 0, all 8 chunks fused: theta' = j*phi'; F = round - theta'
            tha = cp.tile([128, CHUNKS * NW], f32)
            nc.vector.tensor_tensor(
                out=tha[:].rearrange("p (t d j) -> p t d j", t=CHUNKS, d=3),
                in0=it[:, 0 : 3 * CHUNKS]
                .rearrange("p (t d) -> p t d", d=3)
                .unsqueeze(3)
                .broadcast_to([128, CHUNKS, 3, NJ]),
                in1=jt[:, 0:NW]
                .rearrange("p (d j) -> p d j", d=3)
                .unsqueeze(1)
                .broadcast_to([128, CHUNKS, 3, NJ]),
                op=Alu.mult,
            )
            t1a = cp.tile([128, CHUNKS * NW], f32)
            nc.vector.tensor_scalar(
                out=t1a[:], in0=tha[:], scalar1=MAGIC, scalar2=None, op0=Alu.add
            )
            Fa = cp.tile([128, CHUNKS * NW], f32)
            nc.vector.scalar_tensor_tensor(
                out=Fa[:], in0=t1a[:], scalar=-MAGIC, in1=tha[:],
                op0=Alu.add, op1=Alu.subtract,
            )

            Fv = Fa[:].rearrange("p (t w) -> p t w", t=CHUNKS)  # [128, 8, 63]

            for pr in range(CHUNKS // 2):
                tp = slice(2 * pr, 2 * pr + 2)
                # pair sum un = F2 (+) F3 (two chunks): -(psi') mod 1 in [-1,1]
                un = wp.tile([128, 2 * NPAIR], f32)
                nc.gpsimd.tensor_tensor(
                    out=un[:].rearrange("p (c a b) -> p c a b", c=2, a=NJ),
                    in0=Fv[:, tp, NJ : 2 * NJ]
                    .unsqueeze(3)
                    .broadcast_to([128, 2, NJ, NJ]),
                    in1=Fv[:, tp, 2 * NJ : 3 * NJ]
                    .unsqueeze(2)
                    .broadcast_to([128, 2, NJ, NJ]),
                    op=Alu.add,
                )
                # V = all Sin inputs (turns, sign-flipped): [A_i|A_r|c1|s1]
                # blocks are 442 wide (fp32r); col 441 of each is zero pad
                V = wp.tile([128, NV], f32)
                Vb = V[:, 0 : 4 * NPP].rearrange("p (blk w) -> p blk w", blk=4)
                nc.gpsimd.memset(Vb[:, :, NPAIR:NPP], 0.0)
                unv = un[:].rearrange("p (c w) -> p c w", c=2)
                nc.vector.add_range_wrap(
                    out=Vb[:, 0:2, 0:NPAIR], in_=unv, shift=0.0,
                    bound=0.5, period=1.0,
                )
                nc.vector.add_range_wrap(
                    out=Vb[:, 2:4, 0:NPAIR], in_=unv, shift=-0.25,
                    bound=0.5, period=1.0,
                )
                F1p = Fv[:, tp, NK : NK + NH]
                nc.vector.add_range_wrap(
                    out=V[:, 4 * NPP : 4 * NPP + 2 * NH].rearrange(
                        "p (c j) -> p c j", c=2
                    ),
                    in_=F1p, shift=-0.25, bound=0.5, period=1.0,
                )
                nc.vector.tensor_copy(
                    out=V[:, 4 * NPP + 2 * NH : NV].rearrange(
                        "p (c j) -> p c j", c=2
                    ),
                    in_=F1p,
                )
                # Sin(-2pi*v): [A_i c0|A_i c1|A_r c0|A_r c1|c1 tbl|s1 tbl]
                AA = wp.tile([128, NV], f32r)
                nc.scalar.activation(
                    out=AA[:], in_=V[:], func=Act.Sin, bias=0.0, scale=cm2pi
                )
                # lhsT = [q*c1 | q*s1] for both chunks: [128, 44]
                lhsT = wp.tile([128, 4 * NH], f32r)
                nc.vector.tensor_tensor(
                    out=lhsT[:].rearrange("p (c k j) -> p c k j", c=2, k=2),
                    in0=AA[:, 4 * NPP : NV]
                    .rearrange("p (k c j) -> p c k j", k=2, c=2),
                    in1=it[:, 3 * CHUNKS + 2 * pr : 3 * CHUNKS + 2 * pr + 2]
                    .unsqueeze(2)
                    .unsqueeze(3)
                    .broadcast_to([128, 2, 2, NH]),
                    op=Alu.mult,
                )
                for c in range(2):
                    t = 2 * pr + c
                    nc.tensor.matmul(
                        out=ps_i[:],
                        lhsT=lhsT[:, 2 * NH * c : 2 * NH * (c + 1)],
                        rhs=AA[:, NPP * c : NPP * (c + 1)],
                        start=(t == 0), stop=(t == CHUNKS - 1),
                    )
                    nc.tensor.matmul(
                        out=ps_r[:],
                        lhsT=lhsT[:, 2 * NH * c : 2 * NH * (c + 1)],
                        rhs=AA[:, 2 * NPP + NPP * c : 3 * NPP + NPP * c],
                        start=(t == 0), stop=(t == CHUNKS - 1),
                    )

            # PSUM -> SBUF -> DRAM (combine happens on host)
            so = wp.tile([2 * NH, 2 * NPP], f32)
            nc.vector.tensor_copy(out=so[:, 0:NPP], in_=ps_r[:])
            nc.vector.tensor_copy(out=so[:, NPP : 2 * NPP], in_=ps_i[:])
            nc.sync.dma_start(out=sout[:], in_=so[:])

    nc.compile()
    return nc


def _get_nc():
    if "nc" not in _CACHE:
        _CACHE["nc"] = _build_nc()
    return _CACHE["nc"]


def _host_inputs(q, r, cell):
    """Per-core phi (reduced turns) and q in SBUF layout."""
    in_maps = []
    for c in range(N_CORES):
        b = c // CORES_PER_SYS
        half = c % CORES_PER_SYS
        lo = b * N_PER + half * ATOMS_PER_CORE
        rs = r[lo : lo + ATOMS_PER_CORE].astype(np.float64)
        qs = q[lo : lo + ATOMS_PER_CORE, 0].astype(np.float32)
        minv = np.linalg.inv(cell[b].astype(np.float64))
        phi = (rs @ minv) % 1.0                      # [1000, 3] turns in [0,1)
        phi_p = np.zeros((PADN, 3), np.float32)
        phi_p[:ATOMS_PER_CORE] = phi.astype(np.float32)
        q_p = np.zeros((PADN,), np.float32)
        q_p[:ATOMS_PER_CORE] = qs
        # atom (t*128+p) -> [p, t*3+d] and [p, 24+t]
        inp = np.zeros((128, 4 * CHUNKS), np.float32)
        inp[:, 0 : 3 * CHUNKS] = (
            phi_p.reshape(CHUNKS, 128, 3).transpose(1, 0, 2).reshape(128, CHUNKS * 3)
        )
        inp[:, 3 * CHUNKS :] = q_p.reshape(CHUNKS, 128).T
        in_maps.append({"inp": inp})
    return in_maps


def _host_weights(cell):
    """w[b, n1(0..10), n2, n3] = mask * 2 * kfac / V, mirroring reference."""
    k_sq_max = (TWOPI / DL) ** 2
    sigma_sq_half = SIGMA ** 2 / 2.0
    rng = np.arange(-NK, NK + 1, dtype=np.float64)
    n1, n2, n3 = np.meshgrid(rng[NK:], rng, rng, indexing="ij")  # n1 >= 0
    nvec = np.stack([n1.ravel(), n2.ravel(), n3.ravel()], axis=1)  # [NH*441, 3]
    hemi = (
        (nvec[:, 0] > 0)
        | ((nvec[:, 0] == 0) & (nvec[:, 1] > 0))
        | ((nvec[:, 0] == 0) & (nvec[:, 1] == 0) & (nvec[:, 2] > 0))
    )
    ws = []
    for b in range(B):
        cb = cell[b].astype(np.float64)
        G = TWOPI * np.linalg.inv(cb).T
        kvec = nvec @ G
        k_sq = np.sum(kvec ** 2, axis=1)
        mask = (k_sq > 0) & (k_sq <= k_sq_max) & hemi
        kfac = np.exp(-sigma_sq_half * k_sq) / (k_sq + EPS)
        vol = np.linalg.det(cb)
        ws.append(np.where(mask, 2.0 * kfac, 0.0) / vol)
    return np.stack(ws).reshape(B, NH, NPAIR)


def kernel(q, r, cell, batch):
    from concourse.bass_utils import run_bass_kernel_spmd

    q = np.asarray(q)
    r = np.asarray(r)
    cell = np.asarray(cell)

    nc = _get_nc()
    in_maps = _host_inputs(q, r, cell)
    res = run_bass_kernel_spmd(nc, in_maps, core_ids=list(range(N_CORES))).results

    w = _host_weights(cell)
    pot = np.zeros(B, np.float64)
    for b in range(B):
        s_r = np.zeros((NH, NPAIR), np.float64)
        s_i = np.zeros_like(s_r)
        for half in range(CORES_PER_SYS):
            o = res[b * CORES_PER_SYS + half]["sout"].astype(np.float64)
            P, Q = o[0:NH, 0:NPAIR], o[NH : 2 * NH, 0:NPAIR]
            R, T = o[0:NH, NPP : NPP + NPAIR], o[NH : 2 * NH, NPP : NPP + NPAIR]
            s_r += P - T
            s_i += R + Q
        s_sq = s_r ** 2 + s_i ** 2
        qb = q[b * N_PER : (b + 1) * N_PER, 0].astype(np.float64)
        self_e = np.sum(qb ** 2) / (SIGMA * TWOPI ** 1.5)
        pot[b] = (np.sum(w[b] * s_sq) - self_e) * NORM
    return pot.astype(np.float32)


# revision 20
# speedup vs baseline: 1.2197x; 1.1418x over previous
"""Ewald reciprocal-space sum on 8 Trainium2 NeuronCores.

Math: for each system b, S(k) = sum_n q_n e^{i k.r_n} over the static
integer k-grid n in [-10,10]^3, k = n @ G, G = 2*pi*inv(cell)^T.
Key identity: k.r = n1*phi1 + n2*phi2 + n3*phi3 with phi_d = G_d . r,
so e^{i k.r} factorizes into per-dimension phase tables. Only the
n1 >= 0 half-grid is needed (hemisphere mask kills n1 < 0).

Device work per core (SPMD, core c owns half the atoms of system c//2):
  - phases phi'_d = frac((r @ inv(cell))_d) come in pre-reduced (turns)
  - theta'[j,d] = j * phi'_d  for j in [-10..10]        (DVE)
  - reduce mod 1 into [0,1) turn space                  (DVE/GPSIMD)
  - sin/cos via ACT Sin(2*pi*t - pi)                    (ACT)
  - pair table A = e^{i(n2*phi2+n3*phi3)}  [atoms,441]  (ACT)
  - S partial = (q*e^{i n1 phi1})^T @ A via 2 PSUM-accumulated
    matmuls per 128-atom chunk                          (PE)
Host: tiny O(B*K) weight mask + final reduction (exactly mirrors the
reference), summing partial S across the core pair before squaring.
"""

import numpy as np

# ---- problem constants (hardcoded per contract) ----
B = 4
N_PER = 2000
NK = 10                      # k-grid extent: n in [-NK, NK]
NJ = 2 * NK + 1              # 21
NPAIR = NJ * NJ              # 441
NH = NK + 1                  # 11 non-negative n1 values
DL = 2.0
SIGMA = 1.0
EPS = 1e-6
NORM = 90.0474
TWOPI = 2.0 * np.pi

MAGIC = 12582912.0           # 1.5 * 2**23: fp32 round-to-nearest trick
NPP = NPAIR + 1              # 442: fp32r matmul needs even free counts

N_CORES = 8
CORES_PER_SYS = 2
ATOMS_PER_CORE = (B * N_PER) // N_CORES     # 1000
CHUNKS = 8                                  # ceil(1000/128)
PADN = CHUNKS * 128                         # 1024

_CACHE = {}


def _build_nc():
    import concourse.bacc as bacc
    import concourse.mybir as mybir
    import concourse.tile as tile

    f32 = mybir.dt.float32
    Alu = mybir.AluOpType
    Act = mybir.ActivationFunctionType

    f32r = mybir.dt.float32r
    nc = bacc.Bacc(None, target_bir_lowering=False)

    # one input tensor: cols 0:24 = phi (chunk-major, 3/chunk), 24:32 = q
    inp = nc.dram_tensor("inp", [128, 3 * CHUNKS + CHUNKS], f32, kind="ExternalInput")
    sout = nc.dram_tensor("sout", [2 * NH, 2 * NPP], f32, kind="ExternalOutput")

    # j values (d-major blocks of 21, col = d*21 + j+10) + col 63 = -2pi
    jdat = np.concatenate(
        [
            np.tile(np.arange(-NK, NK + 1, dtype=np.float32), (128, 3)),
            np.full((128, 1), -TWOPI, np.float32),
        ],
        axis=1,
    )
    jrow = nc.inline_tensor(jdat, name="jrow")

    NW = 3 * NJ                      # 63 cols per chunk in F/th tiles
    NV = 4 * NPP                     # 1768 cols in fused ACT input per pair
    NT = 2 * CHUNKS * NH             # 176 cols of d1 tables (k-major: c1|s1)

    with tile.TileContext(nc) as tc:
        with (
            tc.tile_pool(name="const", bufs=1) as cp,
            tc.tile_pool(name="work", bufs=3) as wp,
            tc.tile_pool(name="psum", bufs=1, space="PSUM") as pp,
        ):
            jt = cp.tile([128, NW + 1], f32)
            nc.gpsimd.dma_start(out=jt[:], in_=jrow[:])
            cm2pi = jt[:, NW : NW + 1]
            it = cp.tile([128, 4 * CHUNKS], f32)
            nc.sync.dma_start(out=it[:], in_=inp[:])

            ps_r = pp.tile([2 * NH, NPP], f32)
            ps_i = pp.tile([2 * NH, NPP], f32)

            # stage 0, all 8 chunks fused: theta' = j*phi'; F = round - theta'
            tha = cp.tile([128, CHUNKS * NW], f32)
            nc.vector.tensor_tensor(
                out=tha[:].rearrange("p (t d j) -> p t d j", t=CHUNKS, d=3),
                in0=it[:, 0 : 3 * CHUNKS]
                .rearrange("p (t d) -> p t d", d=3)
                .unsqueeze(3)
                .broadcast_to([128, CHUNKS, 3, NJ]),
                in1=jt[:, 0:NW]
                .rearrange("p (d j) -> p d j", d=3)
                .unsqueeze(1)
                .broadcast_to([128, CHUNKS, 3, NJ]),
                op=Alu.mult,
            )
            t1a = cp.tile([128, CHUNKS * NW], f32)
            nc.vector.tensor_scalar(
                out=t1a[:], in0=tha[:], scalar1=MAGIC, scalar2=None, op0=Alu.add
            )
            Fa = cp.tile([128, CHUNKS * NW], f32)
            nc.vector.scalar_tensor_tensor(
                out=Fa[:], in0=t1a[:], scalar=-MAGIC, in1=tha[:],
                op0=Alu.add, op1=Alu.subtract,
            )

            Fv = Fa[:].rearrange("p (t w) -> p t w", t=CHUNKS)  # [128, 8, 63]

            # d1 tables for all chunks: VT = [cos src (8*11) | sin src (8*11)]
            F1a = Fv[:, :, NK : NK + NH]                       # [128, 8, 11]
            VT = cp.tile([128, NT], f32)
            nc.vector.add_range_wrap(
                out=VT[:, 0 : CHUNKS * NH].rearrange("p (t j) -> p t j", t=CHUNKS),
                in_=F1a, shift=-0.25, bound=0.5, period=1.0,
            )
            nc.vector.tensor_copy(
                out=VT[:, CHUNKS * NH : NT].rearrange("p (t j) -> p t j", t=CHUNKS),
                in_=F1a,
            )
            TT = cp.tile([128, NT], f32)
            nc.scalar.activation(
                out=TT[:], in_=VT[:], func=Act.Sin, bias=0.0, scale=cm2pi
            )
            # lhsT_all = [q*c1 | q*s1] per chunk: [128, 8*22], one op
            lhsTa = cp.tile([128, NT], f32r)
            nc.vector.tensor_tensor(
                out=lhsTa[:].rearrange("p (t k j) -> p t k j", t=CHUNKS, k=2),
                in0=TT[:].rearrange("p (k t j) -> p t k j", k=2, t=CHUNKS),
                in1=it[:, 3 * CHUNKS : 4 * CHUNKS]
                .unsqueeze(2)
                .unsqueeze(3)
                .broadcast_to([128, CHUNKS, 2, NH]),
                op=Alu.mult,
            )

            for pr in range(CHUNKS // 2):
                tp = slice(2 * pr, 2 * pr + 2)
                # pair sum un = F2 (+) F3 (two chunks): -(psi') mod 1 in [-1,1]
                un = wp.tile([128, 2 * NPAIR], f32)
                nc.gpsimd.tensor_tensor(
                    out=un[:].rearrange("p (c a b) -> p c a b", c=2, a=NJ),
                    in0=Fv[:, tp, NJ : 2 * NJ]
                    .unsqueeze(3)
                    .broadcast_to([128, 2, NJ, NJ]),
                    in1=Fv[:, tp, 2 * NJ : 3 * NJ]
                    .unsqueeze(2)
                    .broadcast_to([128, 2, NJ, NJ]),
                    op=Alu.add,
                )
                # V = all Sin inputs (turns, sign-flipped): [A_i|A_r|c1|s1]
                # blocks are 442 wide (fp32r); col 441 of each is zero pad
                V = wp.tile([128, NV], f32)
                Vb = V[:, 0 : 4 * NPP].rearrange("p (blk w) -> p blk w", blk=4)
                nc.gpsimd.memset(Vb[:, :, NPAIR:NPP], 0.0)
                unv = un[:].rearrange("p (c w) -> p c w", c=2)
                nc.vector.add_range_wrap(
                    out=Vb[:, 0:2, 0:NPAIR], in_=unv, shift=0.0,
                    bound=0.5, period=1.0,
                )
                nc.vector.add_range_wrap(
                    out=Vb[:, 2:4, 0:NPAIR], in_=unv, shift=-0.25,
                    bound=0.5, period=1.0,
                )
                # Sin(-2pi*v): [A_i c0|A_i c1|A_r c0|A_r c1]; split in two
                # ops so matmuls can start after the first half
                AA = wp.tile([128, NV], f32r)
                nc.scalar.activation(
                    out=AA[:, 0 : 2 * NPP], in_=V[:, 0 : 2 * NPP],
                    func=Act.Sin, bias=0.0, scale=cm2pi,
                )
                nc.scalar.activation(
                    out=AA[:, 2 * NPP : NV], in_=V[:, 2 * NPP : NV],
                    func=Act.Sin, bias=0.0, scale=cm2pi,
                )
                for c in range(2):
                    t = 2 * pr + c
                    lh = lhsTa[:, 2 * NH * t : 2 * NH * (t + 1)]
                    nc.tensor.matmul(
                        out=ps_i[:], lhsT=lh,
                        rhs=AA[:, NPP * c : NPP * (c + 1)],
                        start=(t == 0), stop=(t == CHUNKS - 1),
                    )
                    nc.tensor.matmul(
                        out=ps_r[:], lhsT=lh,
                        rhs=AA[:, 2 * NPP + NPP * c : 3 * NPP + NPP * c],
                        start=(t == 0), stop=(t == CHUNKS - 1),
                    )

            # PSUM -> SBUF -> DRAM (combine happens on host)
            so = wp.tile([2 * NH, 2 * NPP], f32)
            nc.vector.tensor_copy(out=so[:, 0:NPP], in_=ps_r[:])
            nc.scalar.activation(
                out=so[:, NPP : 2 * NPP], in_=ps_i[:], func=Act.Copy
            )
            nc.sync.dma_start(out=sout[:], in_=so[:])

    nc.compile()
    return nc


def _get_nc():
    if "nc" not in _CACHE:
        _CACHE["nc"] = _build_nc()
    return _CACHE["nc"]


def _host_inputs(q, r, cell):
    """Per-core phi (reduced turns) and q in SBUF layout."""
    in_maps = []
    for c in range(N_CORES):
        b = c // CORES_PER_SYS
        half = c % CORES_PER_SYS
        lo = b * N_PER + half * ATOMS_PER_CORE
        rs = r[lo : lo + ATOMS_PER_CORE].astype(np.float64)
        qs = q[lo : lo + ATOMS_PER_CORE, 0].astype(np.float32)
        minv = np.linalg.inv(cell[b].astype(np.float64))
        phi = (rs @ minv) % 1.0                      # [1000, 3] turns in [0,1)
        phi_p = np.zeros((PADN, 3), np.float32)
        phi_p[:ATOMS_PER_CORE] = phi.astype(np.float32)
        q_p = np.zeros((PADN,), np.float32)
        q_p[:ATOMS_PER_CORE] = qs
        # atom (t*128+p) -> [p, t*3+d] and [p, 24+t]
        inp = np.zeros((128, 4 * CHUNKS), np.float32)
        inp[:, 0 : 3 * CHUNKS] = (
            phi_p.reshape(CHUNKS, 128, 3).transpose(1, 0, 2).reshape(128, CHUNKS * 3)
        )
        inp[:, 3 * CHUNKS :] = q_p.reshape(CHUNKS, 128).T
        in_maps.append({"inp": inp})
    return in_maps


def _host_weights(cell):
    """w[b, n1(0..10), n2, n3] = mask * 2 * kfac / V, mirroring reference."""
    k_sq_max = (TWOPI / DL) ** 2
    sigma_sq_half = SIGMA ** 2 / 2.0
    rng = np.arange(-NK, NK + 1, dtype=np.float64)
    n1, n2, n3 = np.meshgrid(rng[NK:], rng, rng, indexing="ij")  # n1 >= 0
    nvec = np.stack([n1.ravel(), n2.ravel(), n3.ravel()], axis=1)  # [NH*441, 3]
    hemi = (
        (nvec[:, 0] > 0)
        | ((nvec[:, 0] == 0) & (nvec[:, 1] > 0))
        | ((nvec[:, 0] == 0) & (nvec[:, 1] == 0) & (nvec[:, 2] > 0))
    )
    ws = []
    for b in range(B):
        cb = cell[b].astype(np.float64)
        G = TWOPI * np.linalg.inv(cb).T
        kvec = nvec @ G
        k_sq = np.sum(kvec ** 2, axis=1)
        mask = (k_sq > 0) & (k_sq <= k_sq_max) & hemi
        kfac = np.exp(-sigma_sq_half * k_sq) / (k_sq + EPS)
        vol = np.linalg.det(cb)
        ws.append(np.where(mask, 2.0 * kfac, 0.0) / vol)
    return np.stack(ws).reshape(B, NH, NPAIR)


def kernel(q, r, cell, batch):
    from concourse.bass_utils import run_bass_kernel_spmd

    q = np.asarray(q)
    r = np.asarray(r)
    cell = np.asarray(cell)

    nc = _get_nc()
    in_maps = _host_inputs(q, r, cell)
    res = run_bass_kernel_spmd(nc, in_maps, core_ids=list(range(N_CORES))).results

    w = _host_weights(cell)
    pot = np.zeros(B, np.float64)
    for b in range(B):
        s_r = np.zeros((NH, NPAIR), np.float64)
        s_i = np.zeros_like(s_r)
        for half in range(CORES_PER_SYS):
            o = res[b * CORES_PER_SYS + half]["sout"].astype(np.float64)
            P, Q = o[0:NH, 0:NPAIR], o[NH : 2 * NH, 0:NPAIR]
            R, T = o[0:NH, NPP : NPP + NPAIR], o[NH : 2 * NH, NPP : NPP + NPAIR]
            s_r += P - T
            s_i += R + Q
        s_sq = s_r ** 2 + s_i ** 2
        qb = q[b * N_PER : (b + 1) * N_PER, 0].astype(np.float64)
        self_e = np.sum(qb ** 2) / (SIGMA * TWOPI ** 1.5)
        pot[b] = (np.sum(w[b] * s_sq) - self_e) * NORM
    return pot.astype(np.float32)


# revision 21
# speedup vs baseline: 1.2318x; 1.0099x over previous
"""Ewald reciprocal-space sum on 8 Trainium2 NeuronCores.

Math: for each system b, S(k) = sum_n q_n e^{i k.r_n} over the static
integer k-grid n in [-10,10]^3, k = n @ G, G = 2*pi*inv(cell)^T.
Key identity: k.r = n1*phi1 + n2*phi2 + n3*phi3 with phi_d = G_d . r,
so e^{i k.r} factorizes into per-dimension phase tables. Only the
n1 >= 0 half-grid is needed (hemisphere mask kills n1 < 0).

Device work per core (SPMD, core c owns half the atoms of system c//2):
  - phases phi'_d = frac((r @ inv(cell))_d) come in pre-reduced (turns)
  - theta'[j,d] = j * phi'_d  for j in [-10..10]        (DVE)
  - reduce mod 1 into [0,1) turn space                  (DVE/GPSIMD)
  - sin/cos via ACT Sin(2*pi*t - pi)                    (ACT)
  - pair table A = e^{i(n2*phi2+n3*phi3)}  [atoms,441]  (ACT)
  - S partial = (q*e^{i n1 phi1})^T @ A via 2 PSUM-accumulated
    matmuls per 128-atom chunk                          (PE)
Host: tiny O(B*K) weight mask + final reduction (exactly mirrors the
reference), summing partial S across the core pair before squaring.
"""

import numpy as np

# ---- problem constants (hardcoded per contract) ----
B = 4
N_PER = 2000
NK = 10                      # k-grid extent: n in [-NK, NK]
NJ = 2 * NK + 1              # 21
NPAIR = NJ * NJ              # 441
NH = NK + 1                  # 11 non-negative n1 values
DL = 2.0
SIGMA = 1.0
EPS = 1e-6
NORM = 90.0474
TWOPI = 2.0 * np.pi

MAGIC = 12582912.0           # 1.5 * 2**23: fp32 round-to-nearest trick
NPP = NPAIR + 1              # 442: fp32r matmul needs even free counts

N_CORES = 8
CORES_PER_SYS = 2
ATOMS_PER_CORE = (B * N_PER) // N_CORES     # 1000
CHUNKS = 8                                  # ceil(1000/128)
PADN = CHUNKS * 128                         # 1024

_CACHE = {}


def _build_nc():
    import concourse.bacc as bacc
    import concourse.mybir as mybir
    import concourse.tile as tile

    f32 = mybir.dt.float32
    Alu = mybir.AluOpType
    Act = mybir.ActivationFunctionType

    f32r = mybir.dt.float32r
    nc = bacc.Bacc(None, target_bir_lowering=False)

    # one input tensor: cols 0:24 = phi (chunk-major, 3/chunk), 24:32 = q
    inp = nc.dram_tensor("inp", [128, 3 * CHUNKS + CHUNKS], f32, kind="ExternalInput")
    sout = nc.dram_tensor("sout", [2 * NH, 2 * NPP], f32, kind="ExternalOutput")

    # j values (d-major blocks of 21, col = d*21 + j+10) + col 63 = -2pi
    jdat = np.concatenate(
        [
            np.tile(np.arange(-NK, NK + 1, dtype=np.float32), (128, 3)),
            np.full((128, 1), -TWOPI, np.float32),
        ],
        axis=1,
    )
    jrow = nc.inline_tensor(jdat, name="jrow")

    NW = 3 * NJ                      # 63 cols per chunk in F/th tiles
    NV = 4 * NPP                     # 1768 cols in fused ACT input per pair
    NT = 2 * CHUNKS * NH             # 176 cols of d1 tables (k-major: c1|s1)

    with tile.TileContext(nc) as tc:
        with (
            tc.tile_pool(name="const", bufs=1) as cp,
            tc.tile_pool(name="work", bufs=3) as wp,
            tc.tile_pool(name="psum", bufs=1, space="PSUM") as pp,
        ):
            jt = cp.tile([128, NW + 1], f32)
            nc.gpsimd.dma_start(out=jt[:], in_=jrow[:])
            cm2pi = jt[:, NW : NW + 1]
            it = cp.tile([128, 4 * CHUNKS], f32)
            nc.sync.dma_start(out=it[:], in_=inp[:])

            ps_r = pp.tile([2 * NH, NPP], f32)
            ps_i = pp.tile([2 * NH, NPP], f32)

            # stage 0, all 8 chunks fused: theta' = j*phi'; F = round - theta'
            tha = cp.tile([128, CHUNKS * NW], f32)
            nc.vector.tensor_tensor(
                out=tha[:].rearrange("p (t d j) -> p t d j", t=CHUNKS, d=3),
                in0=it[:, 0 : 3 * CHUNKS]
                .rearrange("p (t d) -> p t d", d=3)
                .unsqueeze(3)
                .broadcast_to([128, CHUNKS, 3, NJ]),
                in1=jt[:, 0:NW]
                .rearrange("p (d j) -> p d j", d=3)
                .unsqueeze(1)
                .broadcast_to([128, CHUNKS, 3, NJ]),
                op=Alu.mult,
            )
            t1a = cp.tile([128, CHUNKS * NW], f32)
            nc.vector.tensor_scalar(
                out=t1a[:], in0=tha[:], scalar1=MAGIC, scalar2=None, op0=Alu.add
            )
            Fa = cp.tile([128, CHUNKS * NW], f32)
            nc.vector.scalar_tensor_tensor(
                out=Fa[:], in0=t1a[:], scalar=-MAGIC, in1=tha[:],
                op0=Alu.add, op1=Alu.subtract,
            )

            Fv = Fa[:].rearrange("p (t w) -> p t w", t=CHUNKS)  # [128, 8, 63]

            # d1 tables, (t, k, j) interleaved: cols 22t+j = c1, 22t+11+j = s1
            F1a = Fv[:, :, NK : NK + NH]                       # [128, 8, 11]
            VT = cp.tile([128, NT], f32)
            TTv = lambda tile, k: tile[:].rearrange(
                "p (t k j) -> p t k j", t=CHUNKS, k=2
            )[:, :, k, :]
            nc.vector.add_range_wrap(
                out=TTv(VT, 0), in_=F1a, shift=-0.25, bound=0.5, period=1.0
            )
            TT = cp.tile([128, NT], f32)
            nc.scalar.activation(
                out=TTv(TT, 1), in_=F1a, func=Act.Sin, bias=0.0, scale=cm2pi
            )
            nc.scalar.activation(
                out=TTv(TT, 0), in_=TTv(VT, 0), func=Act.Sin, bias=0.0, scale=cm2pi
            )
            # lhsT_all = [q*c1 | q*s1] per chunk: [128, 8*22], one op
            lhsTa = cp.tile([128, NT], f32r)
            nc.vector.tensor_tensor(
                out=lhsTa[:].rearrange("p (t k j) -> p t k j", t=CHUNKS, k=2),
                in0=TT[:].rearrange("p (t k j) -> p t k j", t=CHUNKS, k=2),
                in1=it[:, 3 * CHUNKS : 4 * CHUNKS]
                .unsqueeze(2)
                .unsqueeze(3)
                .broadcast_to([128, CHUNKS, 2, NH]),
                op=Alu.mult,
            )

            for pr in range(CHUNKS // 2):
                tp = slice(2 * pr, 2 * pr + 2)
                # pair sum un = F2 (+) F3 (two chunks): -(psi') mod 1 in [-1,1]
                un = wp.tile([128, 2 * NPAIR], f32)
                nc.gpsimd.tensor_tensor(
                    out=un[:].rearrange("p (c a b) -> p c a b", c=2, a=NJ),
                    in0=Fv[:, tp, NJ : 2 * NJ]
                    .unsqueeze(3)
                    .broadcast_to([128, 2, NJ, NJ]),
                    in1=Fv[:, tp, 2 * NJ : 3 * NJ]
                    .unsqueeze(2)
                    .broadcast_to([128, 2, NJ, NJ]),
                    op=Alu.add,
                )
                # V = all Sin inputs (turns, sign-flipped): [A_i|A_r|c1|s1]
                # blocks are 442 wide (fp32r); col 441 of each is zero pad
                V = wp.tile([128, NV], f32)
                Vb = V[:, 0 : 4 * NPP].rearrange("p (blk w) -> p blk w", blk=4)
                nc.gpsimd.memset(Vb[:, :, NPAIR:NPP], 0.0)
                unv = un[:].rearrange("p (c w) -> p c w", c=2)
                nc.vector.add_range_wrap(
                    out=Vb[:, 0:2, 0:NPAIR], in_=unv, shift=0.0,
                    bound=0.5, period=1.0,
                )
                nc.vector.add_range_wrap(
                    out=Vb[:, 2:4, 0:NPAIR], in_=unv, shift=-0.25,
                    bound=0.5, period=1.0,
                )
                # Sin(-2pi*v): [A_i c0|A_i c1|A_r c0|A_r c1]; split in two
                # ops so matmuls can start after the first half
                AA = wp.tile([128, NV], f32r)
                nc.scalar.activation(
                    out=AA[:, 0 : 2 * NPP], in_=V[:, 0 : 2 * NPP],
                    func=Act.Sin, bias=0.0, scale=cm2pi,
                )
                nc.scalar.activation(
                    out=AA[:, 2 * NPP : NV], in_=V[:, 2 * NPP : NV],
                    func=Act.Sin, bias=0.0, scale=cm2pi,
                )
                for c in range(2):
                    t = 2 * pr + c
                    lh = lhsTa[:, 2 * NH * t : 2 * NH * (t + 1)]
                    nc.tensor.matmul(
                        out=ps_i[:], lhsT=lh,
                        rhs=AA[:, NPP * c : NPP * (c + 1)],
                        start=(t == 0), stop=(t == CHUNKS - 1),
                    )
                    nc.tensor.matmul(
                        out=ps_r[:], lhsT=lh,
                        rhs=AA[:, 2 * NPP + NPP * c : 3 * NPP + NPP * c],
                        start=(t == 0), stop=(t == CHUNKS - 1),
                    )

            # PSUM -> SBUF -> DRAM (combine happens on host)
            so = wp.tile([2 * NH, 2 * NPP], f32)
            nc.vector.tensor_copy(out=so[:, 0:NPP], in_=ps_r[:])
            nc.scalar.activation(
                out=so[:, NPP : 2 * NPP], in_=ps_i[:], func=Act.Copy
            )
            nc.sync.dma_start(out=sout[:], in_=so[:])

    nc.compile()
    return nc


def _get_nc():
    if "nc" not in _CACHE:
        _CACHE["nc"] = _build_nc()
    return _CACHE["nc"]


def _host_inputs(q, r, cell):
    """Per-core phi (reduced turns) and q in SBUF layout."""
    in_maps = []
    for c in range(N_CORES):
        b = c // CORES_PER_SYS
        half = c % CORES_PER_SYS
        lo = b * N_PER + half * ATOMS_PER_CORE
        rs = r[lo : lo + ATOMS_PER_CORE].astype(np.float64)
        qs = q[lo : lo + ATOMS_PER_CORE, 0].astype(np.float32)
        minv = np.linalg.inv(cell[b].astype(np.float64))
        phi = (rs @ minv) % 1.0                      # [1000, 3] turns in [0,1)
        phi_p = np.zeros((PADN, 3), np.float32)
        phi_p[:ATOMS_PER_CORE] = phi.astype(np.float32)
        q_p = np.zeros((PADN,), np.float32)
        q_p[:ATOMS_PER_CORE] = qs
        # atom (t*128+p) -> [p, t*3+d] and [p, 24+t]
        inp = np.zeros((128, 4 * CHUNKS), np.float32)
        inp[:, 0 : 3 * CHUNKS] = (
            phi_p.reshape(CHUNKS, 128, 3).transpose(1, 0, 2).reshape(128, CHUNKS * 3)
        )
        inp[:, 3 * CHUNKS :] = q_p.reshape(CHUNKS, 128).T
        in_maps.append({"inp": inp})
    return in_maps


def _host_weights(cell):
    """w[b, n1(0..10), n2, n3] = mask * 2 * kfac / V, mirroring reference."""
    k_sq_max = (TWOPI / DL) ** 2
    sigma_sq_half = SIGMA ** 2 / 2.0
    rng = np.arange(-NK, NK + 1, dtype=np.float64)
    n1, n2, n3 = np.meshgrid(rng[NK:], rng, rng, indexing="ij")  # n1 >= 0
    nvec = np.stack([n1.ravel(), n2.ravel(), n3.ravel()], axis=1)  # [NH*441, 3]
    hemi = (
        (nvec[:, 0] > 0)
        | ((nvec[:, 0] == 0) & (nvec[:, 1] > 0))
        | ((nvec[:, 0] == 0) & (nvec[:, 1] == 0) & (nvec[:, 2] > 0))
    )
    ws = []
    for b in range(B):
        cb = cell[b].astype(np.float64)
        G = TWOPI * np.linalg.inv(cb).T
        kvec = nvec @ G
        k_sq = np.sum(kvec ** 2, axis=1)
        mask = (k_sq > 0) & (k_sq <= k_sq_max) & hemi
        kfac = np.exp(-sigma_sq_half * k_sq) / (k_sq + EPS)
        vol = np.linalg.det(cb)
        ws.append(np.where(mask, 2.0 * kfac, 0.0) / vol)
    return np.stack(ws).reshape(B, NH, NPAIR)


def kernel(q, r, cell, batch):
    from concourse.bass_utils import run_bass_kernel_spmd

    q = np.asarray(q)
    r = np.asarray(r)
    cell = np.asarray(cell)

    nc = _get_nc()
    in_maps = _host_inputs(q, r, cell)
    res = run_bass_kernel_spmd(nc, in_maps, core_ids=list(range(N_CORES))).results

    w = _host_weights(cell)
    pot = np.zeros(B, np.float64)
    for b in range(B):
        s_r = np.zeros((NH, NPAIR), np.float64)
        s_i = np.zeros_like(s_r)
        for half in range(CORES_PER_SYS):
            o = res[b * CORES_PER_SYS + half]["sout"].astype(np.float64)
            P, Q = o[0:NH, 0:NPAIR], o[NH : 2 * NH, 0:NPAIR]
            R, T = o[0:NH, NPP : NPP + NPAIR], o[NH : 2 * NH, NPP : NPP + NPAIR]
            s_r += P - T
            s_i += R + Q
        s_sq = s_r ** 2 + s_i ** 2
        qb = q[b * N_PER : (b + 1) * N_PER, 0].astype(np.float64)
        self_e = np.sum(qb ** 2) / (SIGMA * TWOPI ** 1.5)
        pot[b] = (np.sum(w[b] * s_sq) - self_e) * NORM
    return pot.astype(np.float32)


# revision 23
# speedup vs baseline: 1.2911x; 1.0482x over previous
"""Ewald reciprocal-space sum on 8 Trainium2 NeuronCores.

Math: for each system b, S(k) = sum_n q_n e^{i k.r_n} over the static
integer k-grid n in [-10,10]^3, k = n @ G, G = 2*pi*inv(cell)^T.
Key identity: k.r = n1*phi1 + n2*phi2 + n3*phi3 with phi_d = G_d . r,
so e^{i k.r} factorizes into per-dimension phase tables. Only the
n1 >= 0 half-grid is needed (hemisphere mask kills n1 < 0).

Device work per core (SPMD, core c owns half the atoms of system c//2):
  - phases phi'_d = frac((r @ inv(cell))_d) come in pre-reduced (turns)
  - theta'[j,d] = j * phi'_d  for j in [-10..10]        (DVE)
  - reduce mod 1 into [0,1) turn space                  (DVE/GPSIMD)
  - sin/cos via ACT Sin(2*pi*t - pi)                    (ACT)
  - pair table A = e^{i(n2*phi2+n3*phi3)}  [atoms,441]  (ACT)
  - S partial = (q*e^{i n1 phi1})^T @ A via 2 PSUM-accumulated
    matmuls per 128-atom chunk                          (PE)
Host: tiny O(B*K) weight mask + final reduction (exactly mirrors the
reference), summing partial S across the core pair before squaring.
"""

import numpy as np

# ---- problem constants (hardcoded per contract) ----
B = 4
N_PER = 2000
NK = 10                      # k-grid extent: n in [-NK, NK]
NJ = 2 * NK + 1              # 21
NPAIR = NJ * NJ              # 441
NH = NK + 1                  # 11 non-negative n1 values
DL = 2.0
SIGMA = 1.0
EPS = 1e-6
NORM = 90.0474
TWOPI = 2.0 * np.pi

MAGIC = 12582912.0           # 1.5 * 2**23: fp32 round-to-nearest trick
NPP = NPAIR + 1              # 442: fp32r matmul needs even free counts

N_CORES = 8
CORES_PER_SYS = 2
ATOMS_PER_CORE = (B * N_PER) // N_CORES     # 1000
CHUNKS = 8                                  # ceil(1000/128)
PADN = CHUNKS * 128                         # 1024

_CACHE = {}


def _build_nc():
    import concourse.bacc as bacc
    import concourse.mybir as mybir
    import concourse.tile as tile

    f32 = mybir.dt.float32
    Alu = mybir.AluOpType
    Act = mybir.ActivationFunctionType

    f32r = mybir.dt.float32r
    nc = bacc.Bacc(None, target_bir_lowering=False)

    # one input tensor: cols 0:24 = phi (chunk-major, 3/chunk), 24:32 = q
    inp = nc.dram_tensor("inp", [128, 3 * CHUNKS + CHUNKS], f32, kind="ExternalInput")
    sout = nc.dram_tensor("sout", [2 * NH, 2 * NPP], f32, kind="ExternalOutput")

    # j values (d-major blocks of 21, col = d*21 + j+10) + col 63 = -2pi
    jdat = np.concatenate(
        [
            np.tile(np.arange(-NK, NK + 1, dtype=np.float32), (128, 3)),
            np.full((128, 1), -TWOPI, np.float32),
        ],
        axis=1,
    )
    jrow = nc.inline_tensor(jdat, name="jrow")

    NW = 3 * NJ                      # 63 cols per chunk in F/th tiles
    NV = 4 * NPP                     # 1768 cols in fused ACT input per pair
    NT = 2 * CHUNKS * NH             # 176 cols of d1 tables (k-major: c1|s1)

    with tile.TileContext(nc) as tc:
        with (
            tc.tile_pool(name="const", bufs=1) as cp,
            tc.tile_pool(name="work", bufs=3) as wp,
            tc.tile_pool(name="psum", bufs=1, space="PSUM") as pp,
        ):
            jt = cp.tile([128, NW + 1], f32)
            nc.scalar.dma_start(out=jt[:], in_=jrow[:])
            cm2pi = jt[:, NW : NW + 1]
            it = cp.tile([128, 4 * CHUNKS], f32)
            nc.gpsimd.dma_start(out=it[:], in_=inp[:])

            ps_r = pp.tile([2 * NH, NPP], f32)
            ps_i = pp.tile([2 * NH, NPP], f32)

            # stage 0, all 8 chunks fused: theta' = j*phi'; F = round - theta'
            tha = cp.tile([128, CHUNKS * NW], f32)
            nc.vector.tensor_tensor(
                out=tha[:].rearrange("p (t d j) -> p t d j", t=CHUNKS, d=3),
                in0=it[:, 0 : 3 * CHUNKS]
                .rearrange("p (t d) -> p t d", d=3)
                .unsqueeze(3)
                .broadcast_to([128, CHUNKS, 3, NJ]),
                in1=jt[:, 0:NW]
                .rearrange("p (d j) -> p d j", d=3)
                .unsqueeze(1)
                .broadcast_to([128, CHUNKS, 3, NJ]),
                op=Alu.mult,
            )
            t1a = cp.tile([128, CHUNKS * NW], f32)
            nc.vector.tensor_scalar(
                out=t1a[:], in0=tha[:], scalar1=MAGIC, scalar2=None, op0=Alu.add
            )
            Fa = cp.tile([128, CHUNKS * NW], f32)
            nc.vector.scalar_tensor_tensor(
                out=Fa[:], in0=t1a[:], scalar=-MAGIC, in1=tha[:],
                op0=Alu.add, op1=Alu.subtract,
            )

            Fv = Fa[:].rearrange("p (t w) -> p t w", t=CHUNKS)  # [128, 8, 63]

            # d1 tables, (t, k, j) interleaved: cols 22t+j = c1, 22t+11+j = s1
            F1a = Fv[:, :, NK : NK + NH]                       # [128, 8, 11]
            VT = cp.tile([128, NT], f32)
            TTv = lambda tile, k: tile[:].rearrange(
                "p (t k j) -> p t k j", t=CHUNKS, k=2
            )[:, :, k, :]
            nc.vector.add_range_wrap(
                out=TTv(VT, 0), in_=F1a, shift=-0.25, bound=0.5, period=1.0
            )
            TT = cp.tile([128, NT], f32)
            nc.scalar.activation(
                out=TTv(TT, 1), in_=F1a, func=Act.Sin, bias=0.0, scale=cm2pi
            )
            nc.scalar.activation(
                out=TTv(TT, 0), in_=TTv(VT, 0), func=Act.Sin, bias=0.0, scale=cm2pi
            )
            # lhsT_all = [q*c1 | q*s1] per chunk: [128, 8*22], one op
            lhsTa = cp.tile([128, NT], f32r)
            nc.gpsimd.tensor_tensor(
                out=lhsTa[:].rearrange("p (t k j) -> p t k j", t=CHUNKS, k=2),
                in0=TT[:].rearrange("p (t k j) -> p t k j", t=CHUNKS, k=2),
                in1=it[:, 3 * CHUNKS : 4 * CHUNKS]
                .unsqueeze(2)
                .unsqueeze(3)
                .broadcast_to([128, CHUNKS, 2, NH]),
                op=Alu.mult,
            )

            for pr in range(CHUNKS // 2):
                tp = slice(2 * pr, 2 * pr + 2)
                # pair sum un = F2 (+) F3 (two chunks): -(psi') mod 1 in [-1,1]
                un = wp.tile([128, 2 * NPAIR], f32)
                nc.gpsimd.tensor_tensor(
                    out=un[:].rearrange("p (c a b) -> p c a b", c=2, a=NJ),
                    in0=Fv[:, tp, NJ : 2 * NJ]
                    .unsqueeze(3)
                    .broadcast_to([128, 2, NJ, NJ]),
                    in1=Fv[:, tp, 2 * NJ : 3 * NJ]
                    .unsqueeze(2)
                    .broadcast_to([128, 2, NJ, NJ]),
                    op=Alu.add,
                )
                # V = all Sin inputs (turns, sign-flipped): [A_i|A_r|c1|s1]
                # blocks are 442 wide (fp32r); col 441 of each is zero pad
                V = wp.tile([128, NV], f32)
                Vb = V[:, 0 : 4 * NPP].rearrange("p (blk w) -> p blk w", blk=4)
                nc.gpsimd.memset(Vb[:, :, NPAIR:NPP], 0.0)
                unv = un[:].rearrange("p (c w) -> p c w", c=2)
                nc.vector.add_range_wrap(
                    out=Vb[:, 0:2, 0:NPAIR], in_=unv, shift=0.0,
                    bound=0.5, period=1.0,
                )
                nc.vector.add_range_wrap(
                    out=Vb[:, 2:4, 0:NPAIR], in_=unv, shift=-0.25,
                    bound=0.5, period=1.0,
                )
                # Sin(-2pi*v): [A_i c0|A_i c1|A_r c0|A_r c1]; split in two
                # ops so matmuls can start after the first half
                AA = wp.tile([128, NV], f32r)
                nc.scalar.activation(
                    out=AA[:, 0 : 2 * NPP], in_=V[:, 0 : 2 * NPP],
                    func=Act.Sin, bias=0.0, scale=cm2pi,
                )
                nc.scalar.activation(
                    out=AA[:, 2 * NPP : NV], in_=V[:, 2 * NPP : NV],
                    func=Act.Sin, bias=0.0, scale=cm2pi,
                )
                for c in range(2):
                    t = 2 * pr + c
                    lh = lhsTa[:, 2 * NH * t : 2 * NH * (t + 1)]
                    nc.tensor.matmul(
                        out=ps_i[:], lhsT=lh,
                        rhs=AA[:, NPP * c : NPP * (c + 1)],
                        start=(t == 0), stop=(t == CHUNKS - 1),
                    )
                    nc.tensor.matmul(
                        out=ps_r[:], lhsT=lh,
                        rhs=AA[:, 2 * NPP + NPP * c : 3 * NPP + NPP * c],
                        start=(t == 0), stop=(t == CHUNKS - 1),
                    )

            # PSUM -> SBUF -> DRAM (combine happens on host)
            so = wp.tile([2 * NH, 2 * NPP], f32)
            nc.vector.tensor_copy(out=so[:, 0:NPP], in_=ps_r[:])
            nc.scalar.activation(
                out=so[:, NPP : 2 * NPP], in_=ps_i[:], func=Act.Copy
            )
            nc.sync.dma_start(out=sout[:], in_=so[:])

    nc.compile()
    return nc


def _get_nc():
    if "nc" not in _CACHE:
        _CACHE["nc"] = _build_nc()
    return _CACHE["nc"]


def _host_inputs(q, r, cell):
    """Per-core phi (reduced turns) and q in SBUF layout."""
    in_maps = []
    for c in range(N_CORES):
        b = c // CORES_PER_SYS
        half = c % CORES_PER_SYS
        lo = b * N_PER + half * ATOMS_PER_CORE
        rs = r[lo : lo + ATOMS_PER_CORE].astype(np.float64)
        qs = q[lo : lo + ATOMS_PER_CORE, 0].astype(np.float32)
        minv = np.linalg.inv(cell[b].astype(np.float64))
        phi = (rs @ minv) % 1.0                      # [1000, 3] turns in [0,1)
        phi_p = np.zeros((PADN, 3), np.float32)
        phi_p[:ATOMS_PER_CORE] = phi.astype(np.float32)
        q_p = np.zeros((PADN,), np.float32)
        q_p[:ATOMS_PER_CORE] = qs
        # atom (t*128+p) -> [p, t*3+d] and [p, 24+t]
        inp = np.zeros((128, 4 * CHUNKS), np.float32)
        inp[:, 0 : 3 * CHUNKS] = (
            phi_p.reshape(CHUNKS, 128, 3).transpose(1, 0, 2).reshape(128, CHUNKS * 3)
        )
        inp[:, 3 * CHUNKS :] = q_p.reshape(CHUNKS, 128).T
        in_maps.append({"inp": inp})
    return in_maps


def _host_weights(cell):
    """w[b, n1(0..10), n2, n3] = mask * 2 * kfac / V, mirroring reference."""
    k_sq_max = (TWOPI / DL) ** 2
    sigma_sq_half = SIGMA ** 2 / 2.0
    rng = np.arange(-NK, NK + 1, dtype=np.float64)
    n1, n2, n3 = np.meshgrid(rng[NK:], rng, rng, indexing="ij")  # n1 >= 0
    nvec = np.stack([n1.ravel(), n2.ravel(), n3.ravel()], axis=1)  # [NH*441, 3]
    hemi = (
        (nvec[:, 0] > 0)
        | ((nvec[:, 0] == 0) & (nvec[:, 1] > 0))
        | ((nvec[:, 0] == 0) & (nvec[:, 1] == 0) & (nvec[:, 2] > 0))
    )
    ws = []
    for b in range(B):
        cb = cell[b].astype(np.float64)
        G = TWOPI * np.linalg.inv(cb).T
        kvec = nvec @ G
        k_sq = np.sum(kvec ** 2, axis=1)
        mask = (k_sq > 0) & (k_sq <= k_sq_max) & hemi
        kfac = np.exp(-sigma_sq_half * k_sq) / (k_sq + EPS)
        vol = np.linalg.det(cb)
        ws.append(np.where(mask, 2.0 * kfac, 0.0) / vol)
    return np.stack(ws).reshape(B, NH, NPAIR)


def kernel(q, r, cell, batch):
    from concourse.bass_utils import run_bass_kernel_spmd

    q = np.asarray(q)
    r = np.asarray(r)
    cell = np.asarray(cell)

    nc = _get_nc()
    in_maps = _host_inputs(q, r, cell)
    res = run_bass_kernel_spmd(nc, in_maps, core_ids=list(range(N_CORES))).results

    w = _host_weights(cell)
    pot = np.zeros(B, np.float64)
    for b in range(B):
        s_r = np.zeros((NH, NPAIR), np.float64)
        s_i = np.zeros_like(s_r)
        for half in range(CORES_PER_SYS):
            o = res[b * CORES_PER_SYS + half]["sout"].astype(np.float64)
            P, Q = o[0:NH, 0:NPAIR], o[NH : 2 * NH, 0:NPAIR]
            R, T = o[0:NH, NPP : NPP + NPAIR], o[NH : 2 * NH, NPP : NPP + NPAIR]
            s_r += P - T
            s_i += R + Q
        s_sq = s_r ** 2 + s_i ** 2
        qb = q[b * N_PER : (b + 1) * N_PER, 0].astype(np.float64)
        self_e = np.sum(qb ** 2) / (SIGMA * TWOPI ** 1.5)
        pot[b] = (np.sum(w[b] * s_sq) - self_e) * NORM
    return pot.astype(np.float32)


# revision 24
# speedup vs baseline: 1.2967x; 1.0043x over previous
"""Ewald reciprocal-space sum on 8 Trainium2 NeuronCores.

Math: for each system b, S(k) = sum_n q_n e^{i k.r_n} over the static
integer k-grid n in [-10,10]^3, k = n @ G, G = 2*pi*inv(cell)^T.
Key identity: k.r = n1*phi1 + n2*phi2 + n3*phi3 with phi_d = G_d . r,
so e^{i k.r} factorizes into per-dimension phase tables. Only the
n1 >= 0 half-grid is needed (hemisphere mask kills n1 < 0).

Device work per core (SPMD, core c owns half the atoms of system c//2):
  - phases phi'_d = frac((r @ inv(cell))_d) come in pre-reduced (turns)
  - theta'[j,d] = j * phi'_d  for j in [-10..10]        (DVE)
  - reduce mod 1 into [0,1) turn space                  (DVE/GPSIMD)
  - sin/cos via ACT Sin(2*pi*t - pi)                    (ACT)
  - pair table A = e^{i(n2*phi2+n3*phi3)}  [atoms,441]  (ACT)
  - S partial = (q*e^{i n1 phi1})^T @ A via 2 PSUM-accumulated
    matmuls per 128-atom chunk                          (PE)
Host: tiny O(B*K) weight mask + final reduction (exactly mirrors the
reference), summing partial S across the core pair before squaring.
"""

import numpy as np

# ---- problem constants (hardcoded per contract) ----
B = 4
N_PER = 2000
NK = 10                      # k-grid extent: n in [-NK, NK]
NJ = 2 * NK + 1              # 21
NPAIR = NJ * NJ              # 441
NH = NK + 1                  # 11 non-negative n1 values
DL = 2.0
SIGMA = 1.0
EPS = 1e-6
NORM = 90.0474
TWOPI = 2.0 * np.pi

MAGIC = 12582912.0           # 1.5 * 2**23: fp32 round-to-nearest trick
NPP = NPAIR + 1              # 442: fp32r matmul needs even free counts

N_CORES = 8
CORES_PER_SYS = 2
ATOMS_PER_CORE = (B * N_PER) // N_CORES     # 1000
CHUNKS = 8                                  # ceil(1000/128)
PADN = CHUNKS * 128                         # 1024

_CACHE = {}


def _build_nc():
    import concourse.bacc as bacc
    import concourse.mybir as mybir
    import concourse.tile as tile

    f32 = mybir.dt.float32
    Alu = mybir.AluOpType
    Act = mybir.ActivationFunctionType

    f32r = mybir.dt.float32r
    nc = bacc.Bacc(None, target_bir_lowering=False)

    # one input tensor: cols 0:24 = phi (chunk-major, 3/chunk), 24:32 = q
    inp = nc.dram_tensor("inp", [128, 3 * CHUNKS + CHUNKS], f32, kind="ExternalInput")
    sout = nc.dram_tensor("sout", [2 * NH, 2 * NPP], f32, kind="ExternalOutput")

    # j values (d-major blocks of 21, col = d*21 + j+10) + col 63 = -2pi
    jdat = np.concatenate(
        [
            np.tile(np.arange(-NK, NK + 1, dtype=np.float32), (128, 3)),
            np.full((128, 1), -TWOPI, np.float32),
        ],
        axis=1,
    )
    jrow = nc.inline_tensor(jdat, name="jrow")

    NW = 3 * NJ                      # 63 cols per chunk in F/th tiles
    NV = 4 * NPP                     # 1768 cols in fused ACT input per pair
    NT = 2 * CHUNKS * NH             # 176 cols of d1 tables (k-major: c1|s1)

    with tile.TileContext(nc) as tc:
        with (
            tc.tile_pool(name="const", bufs=1) as cp,
            tc.tile_pool(name="work", bufs=3) as wp,
            tc.tile_pool(name="psum", bufs=1, space="PSUM") as pp,
        ):
            jt = cp.tile([128, NW + 1], f32)
            nc.scalar.dma_start(out=jt[:], in_=jrow[:])
            cm2pi = jt[:, NW : NW + 1]
            it = cp.tile([128, 4 * CHUNKS], f32)
            nc.scalar.dma_start(out=it[:], in_=inp[:])

            ps_r = pp.tile([2 * NH, NPP], f32)
            ps_i = pp.tile([2 * NH, NPP], f32)

            # stage 0 in two halves: theta' = j*phi'; F = round - theta'
            HC = CHUNKS // 2
            tha = cp.tile([128, CHUNKS * NW], f32)
            t1a = cp.tile([128, CHUNKS * NW], f32)
            Fa = cp.tile([128, CHUNKS * NW], f32)
            for h in range(2):
                hs, he = h * HC * NW, (h + 1) * HC * NW
                nc.vector.tensor_tensor(
                    out=tha[:, hs:he].rearrange("p (t d j) -> p t d j", t=HC, d=3),
                    in0=it[:, 3 * h * HC : 3 * (h + 1) * HC]
                    .rearrange("p (t d) -> p t d", d=3)
                    .unsqueeze(3)
                    .broadcast_to([128, HC, 3, NJ]),
                    in1=jt[:, 0:NW]
                    .rearrange("p (d j) -> p d j", d=3)
                    .unsqueeze(1)
                    .broadcast_to([128, HC, 3, NJ]),
                    op=Alu.mult,
                )
                nc.vector.tensor_scalar(
                    out=t1a[:, hs:he], in0=tha[:, hs:he], scalar1=MAGIC,
                    scalar2=None, op0=Alu.add,
                )
                nc.vector.scalar_tensor_tensor(
                    out=Fa[:, hs:he], in0=t1a[:, hs:he], scalar=-MAGIC,
                    in1=tha[:, hs:he], op0=Alu.add, op1=Alu.subtract,
                )

            Fv = Fa[:].rearrange("p (t w) -> p t w", t=CHUNKS)  # [128, 8, 63]

            # d1 tables, (t, k, j) interleaved: cols 22t+j = c1, 22t+11+j = s1
            F1a = Fv[:, :, NK : NK + NH]                       # [128, 8, 11]
            VT = cp.tile([128, NT], f32)
            TT = cp.tile([128, NT], f32)
            lhsTa = cp.tile([128, NT], f32r)
            tkj = lambda ap: ap.rearrange("p (t k j) -> p t k j", t=CHUNKS, k=2)
            nc.vector.add_range_wrap(
                out=tkj(VT[:])[:, :, 0, :], in_=F1a, shift=-0.25,
                bound=0.5, period=1.0,
            )
            nc.scalar.activation(
                out=tkj(TT[:])[:, :, 1, :], in_=F1a, func=Act.Sin,
                bias=0.0, scale=cm2pi,
            )
            nc.scalar.activation(
                out=tkj(TT[:])[:, :, 0, :], in_=tkj(VT[:])[:, :, 0, :],
                func=Act.Sin, bias=0.0, scale=cm2pi,
            )
            nc.gpsimd.tensor_tensor(
                out=tkj(lhsTa[:]),
                in0=tkj(TT[:]),
                in1=it[:, 3 * CHUNKS : 4 * CHUNKS]
                .unsqueeze(2)
                .unsqueeze(3)
                .broadcast_to([128, CHUNKS, 2, NH]),
                op=Alu.mult,
            )

            for t in range(CHUNKS):
                # un = F2 (+) F3: -(psi') mod 1, in [-1, 1]
                un = wp.tile([128, NPAIR], f32)
                nc.gpsimd.tensor_tensor(
                    out=un[:].rearrange("p (a b) -> p a b", a=NJ),
                    in0=Fv[:, t, NJ : 2 * NJ]
                    .unsqueeze(2)
                    .broadcast_to([128, NJ, NJ]),
                    in1=Fv[:, t, 2 * NJ : 3 * NJ]
                    .unsqueeze(1)
                    .broadcast_to([128, NJ, NJ]),
                    op=Alu.add,
                )
                # V = [A_i src (442) | A_r src (442)], col 441 zero pad
                V = wp.tile([128, 2 * NPP], f32)
                Vb = V[:].rearrange("p (blk w) -> p blk w", blk=2)
                nc.gpsimd.memset(Vb[:, :, NPAIR:NPP], 0.0)
                nc.vector.add_range_wrap(
                    out=Vb[:, 0, 0:NPAIR], in_=un[:], shift=0.0,
                    bound=0.5, period=1.0,
                )
                nc.vector.add_range_wrap(
                    out=Vb[:, 1, 0:NPAIR], in_=un[:], shift=-0.25,
                    bound=0.5, period=1.0,
                )
                # Sin(-2pi*v) -> [A_i | A_r]
                AA = wp.tile([128, 2 * NPP], f32r)
                nc.scalar.activation(
                    out=AA[:], in_=V[:], func=Act.Sin, bias=0.0, scale=cm2pi
                )
                lh = lhsTa[:, 2 * NH * t : 2 * NH * (t + 1)]
                nc.tensor.matmul(
                    out=ps_i[:], lhsT=lh, rhs=AA[:, 0:NPP],
                    start=(t == 0), stop=(t == CHUNKS - 1),
                )
                nc.tensor.matmul(
                    out=ps_r[:], lhsT=lh, rhs=AA[:, NPP : 2 * NPP],
                    start=(t == 0), stop=(t == CHUNKS - 1),
                )

            # PSUM -> SBUF -> DRAM (combine happens on host)
            so = wp.tile([2 * NH, 2 * NPP], f32)
            nc.vector.tensor_copy(out=so[:, 0:NPP], in_=ps_r[:])
            nc.scalar.activation(
                out=so[:, NPP : 2 * NPP], in_=ps_i[:], func=Act.Copy
            )
            nc.sync.dma_start(out=sout[:, 0:NPP], in_=so[:, 0:NPP])
            nc.sync.dma_start(out=sout[:, NPP : 2 * NPP], in_=so[:, NPP : 2 * NPP])

    nc.compile()
    return nc


def _get_nc():
    if "nc" not in _CACHE:
        _CACHE["nc"] = _build_nc()
    return _CACHE["nc"]


def _host_inputs(q, r, cell):
    """Per-core phi (reduced turns) and q in SBUF layout."""
    in_maps = []
    for c in range(N_CORES):
        b = c // CORES_PER_SYS
        half = c % CORES_PER_SYS
        lo = b * N_PER + half * ATOMS_PER_CORE
        rs = r[lo : lo + ATOMS_PER_CORE].astype(np.float64)
        qs = q[lo : lo + ATOMS_PER_CORE, 0].astype(np.float32)
        minv = np.linalg.inv(cell[b].astype(np.float64))
        phi = (rs @ minv) % 1.0                      # [1000, 3] turns in [0,1)
        phi_p = np.zeros((PADN, 3), np.float32)
        phi_p[:ATOMS_PER_CORE] = phi.astype(np.float32)
        q_p = np.zeros((PADN,), np.float32)
        q_p[:ATOMS_PER_CORE] = qs
        # atom (t*128+p) -> [p, t*3+d] and [p, 24+t]
        inp = np.zeros((128, 4 * CHUNKS), np.float32)
        inp[:, 0 : 3 * CHUNKS] = (
            phi_p.reshape(CHUNKS, 128, 3).transpose(1, 0, 2).reshape(128, CHUNKS * 3)
        )
        inp[:, 3 * CHUNKS :] = q_p.reshape(CHUNKS, 128).T
        in_maps.append({"inp": inp})
    return in_maps


def _host_weights(cell):
    """w[b, n1(0..10), n2, n3] = mask * 2 * kfac / V, mirroring reference."""
    k_sq_max = (TWOPI / DL) ** 2
    sigma_sq_half = SIGMA ** 2 / 2.0
    rng = np.arange(-NK, NK + 1, dtype=np.float64)
    n1, n2, n3 = np.meshgrid(rng[NK:], rng, rng, indexing="ij")  # n1 >= 0
    nvec = np.stack([n1.ravel(), n2.ravel(), n3.ravel()], axis=1)  # [NH*441, 3]
    hemi = (
        (nvec[:, 0] > 0)
        | ((nvec[:, 0] == 0) & (nvec[:, 1] > 0))
        | ((nvec[:, 0] == 0) & (nvec[:, 1] == 0) & (nvec[:, 2] > 0))
    )
    ws = []
    for b in range(B):
        cb = cell[b].astype(np.float64)
        G = TWOPI * np.linalg.inv(cb).T
        kvec = nvec @ G
        k_sq = np.sum(kvec ** 2, axis=1)
        mask = (k_sq > 0) & (k_sq <= k_sq_max) & hemi
        kfac = np.exp(-sigma_sq_half * k_sq) / (k_sq + EPS)
        vol = np.linalg.det(cb)
        ws.append(np.where(mask, 2.0 * kfac, 0.0) / vol)
    return np.stack(ws).reshape(B, NH, NPAIR)


def kernel(q, r, cell, batch):
    from concourse.bass_utils import run_bass_kernel_spmd

    q = np.asarray(q)
    r = np.asarray(r)
    cell = np.asarray(cell)

    nc = _get_nc()
    in_maps = _host_inputs(q, r, cell)
    res = run_bass_kernel_spmd(nc, in_maps, core_ids=list(range(N_CORES))).results

    w = _host_weights(cell)
    pot = np.zeros(B, np.float64)
    for b in range(B):
        s_r = np.zeros((NH, NPAIR), np.float64)
        s_i = np.zeros_like(s_r)
        for half in range(CORES_PER_SYS):
            o = res[b * CORES_PER_SYS + half]["sout"].astype(np.float64)
            P, Q = o[0:NH, 0:NPAIR], o[NH : 2 * NH, 0:NPAIR]
            R, T = o[0:NH, NPP : NPP + NPAIR], o[NH : 2 * NH, NPP : NPP + NPAIR]
            s_r += P - T
            s_i += R + Q
        s_sq = s_r ** 2 + s_i ** 2
        qb = q[b * N_PER : (b + 1) * N_PER, 0].astype(np.float64)
        self_e = np.sum(qb ** 2) / (SIGMA * TWOPI ** 1.5)
        pot[b] = (np.sum(w[b] * s_sq) - self_e) * NORM
    return pot.astype(np.float32)
